# revision 17
# baseline (speedup 1.0000x reference)
"""Trainium2 Bass kernel for COIL-style pooling head.

Computes, per example:
  cls_rep = LN(hidden[:,0] @ cls_w + cls_b) * g_cls + b_cls            [B, 768]
  reps    = relu(LN(hidden @ tok_w + tok_b) * g_tok + b_tok)           [B, S, 128]
  reps    = sliding-window (w=5) masked mean over compacted tokens 1..S-2,
            then L2-normalized                                          [B, S-2, 128]

Sharding: pure data parallel, batch 32 -> 4 examples on each of 8 cores.

Device pipeline per example (layout [d, s] = token-feature on partitions):
  1. DMA hidden s-tiles naturally [s,h], PE-transpose to [h,s] (fp32).
  2. fp32 matmul accumulation over 6 h-chunks -> z [d=128, s<=512] in PSUM.
  3. LN stats over d via ones-matmul (replicated column sums), center/scale,
     relu (*g on ACT with per-partition scale), mask multiply.
  4. One-op prefix scan along s (tensor_tensor_scan), shifted subtract ->
     10-wide window sums.
  5. PE-transpose back to [s, d], L2 normalize with safe divide + validity
     mask, DMA out.
"""

import sys

sys.path.insert(0, "/opt/trn_rl_repo")

from contextlib import ExitStack

import ml_dtypes
import numpy as np

import concourse.bass as bass
import concourse.tile as tile
from concourse import bacc
from concourse import mybir

B, S_FULL, H, D, CLS = 32, 2048, 768, 128, 768
WIN = 5
EPS = 1e-5
NCORES = 8
BL = B // NCORES  # examples per core

f32 = mybir.dt.float32
f32r = mybir.dt.float32r
bf16 = mybir.dt.bfloat16
AF = mybir.ActivationFunctionType
OP = mybir.AluOpType


def build_program(bl=BL, s=S_FULL):
    """Build the single-core Bass/Tile program (SPMD: same program on all cores)."""
    t_len = s - 2           # compact token count
    nt = s // 128           # s-tiles of 128 tokens
    nch = s // 512          # 512-wide chunks
    ng = nt // 4            # normalize groups of 4 s-tiles
    kh = H // 128           # h chunks (6)

    nc = bacc.Bacc("TRN2")

    # ---- DRAM parameters ----
    hid_h = nc.declare_dram_parameter("hidden", [bl, s, H], f32r, isOutput=False)
    maskbf_h = nc.declare_dram_parameter("maskbf", [1, bl * s], bf16, isOutput=False)
    maskT_h = nc.declare_dram_parameter("maskT", [128, bl * nt], f32, isOutput=False)
    tokw_h = nc.declare_dram_parameter("tok_w", [H, D], f32r, isOutput=False)
    gcol_h = nc.declare_dram_parameter("g_col", [128, 1], f32, isOutput=False)
    clsw_h = nc.declare_dram_parameter("cls_w", [H, CLS], f32, isOutput=False)
    clsb_h = nc.declare_dram_parameter("clsb_rep", [bl, CLS], f32, isOutput=False)
    grep_h = nc.declare_dram_parameter("g_rep", [bl, CLS], f32, isOutput=False)
    brep_h = nc.declare_dram_parameter("b_rep", [bl, CLS], f32, isOutput=False)
    ident_h = nc.declare_dram_parameter("ident", [128, 128], f32, isOutput=False)
    identr_h = nc.declare_dram_parameter("identr", [128, 128], f32r, isOutput=False)
    ones_h = nc.declare_dram_parameter("ones128", [128, 128], f32r, isOutput=False)
    ones1b_h = nc.declare_dram_parameter("ones1b", [1, 128], bf16, isOutput=False)

    cls_out_h = nc.declare_dram_parameter("cls_out", [bl, CLS], f32, isOutput=True)
    reps_out_h = nc.declare_dram_parameter("reps_out", [bl, t_len, D], f32, isOutput=True)

    with tile.TileContext(nc) as tc, ExitStack() as ctx:
        # ---- constant pools (resident) ----
        cpool = ctx.enter_context(tc.tile_pool(name="consts", bufs=1))

        ident_sb = cpool.tile([128, 128], f32, tag="ident")
        nc.sync.dma_start(out=ident_sb[:], in_=ident_h.ap())
        identr_sb = cpool.tile([128, 128], f32r, tag="identr")
        nc.sync.dma_start(out=identr_sb[:], in_=identr_h.ap())
        ones_sb = cpool.tile([128, 128], f32r, tag="ones")
        nc.sync.dma_start(out=ones_sb[:], in_=ones_h.ap())
        ones1b_sb = cpool.tile([1, 128], bf16, tag="ones1b")
        nc.sync.dma_start(out=ones1b_sb[:], in_=ones1b_h.ap())
        gcol_sb = cpool.tile([128, 1], f32, tag="gcol")
        nc.sync.dma_start(out=gcol_sb[:], in_=gcol_h.ap())
        maskbf_sb = cpool.tile([1, bl * s], bf16, tag="maskbf")
        nc.sync.dma_start(out=maskbf_sb[:], in_=maskbf_h.ap())
        maskT_sb = cpool.tile([128, bl * nt], f32, tag="maskT")
        nc.sync.dma_start(out=maskT_sb[:], in_=maskT_h.ap())

        tokw_sb = []
        for k in range(kh):
            w = cpool.tile([128, D], f32r, tag=f"tokw{k}")
            nc.sync.dma_start(out=w[:], in_=tokw_h.ap()[128 * k : 128 * (k + 1), :])
            tokw_sb.append(w)
        clsw_sb = []
        for k in range(kh):
            w = cpool.tile([128, CLS], f32, tag=f"clsw{k}")
            nc.sync.dma_start(out=w[:], in_=clsw_h.ap()[128 * k : 128 * (k + 1), :])
            clsw_sb.append(w)
        clsb_sb = cpool.tile([bl, CLS], f32, tag="clsb")
        nc.sync.dma_start(out=clsb_sb[:], in_=clsb_h.ap())
        grep_sb = cpool.tile([bl, CLS], f32, tag="grep")
        nc.sync.dma_start(out=grep_sb[:], in_=grep_h.ap())
        brep_sb = cpool.tile([bl, CLS], f32, tag="brep")
        nc.sync.dma_start(out=brep_sb[:], in_=brep_h.ap())

        # cls gather buffer: column k*bl+e = h-chunk k of token 0 of example e
        clsx_sb = cpool.tile([128, kh * bl], f32, tag="clsx")

        # bias constants for ACT ops
        eps_col = cpool.tile([128, 1], f32, tag="epsc")
        nc.gpsimd.memset(eps_col[:], EPS)
        tiny_col = cpool.tile([128, 1], f32, tag="tinyc")
        nc.gpsimd.memset(tiny_col[:], 1e-38)

        # ---- working pools ----
        hid_pool = ctx.enter_context(tc.tile_pool(name="hid", bufs=6))
        hidT_pool = ctx.enter_context(tc.tile_pool(name="hidT", bufs=12))
        zsb_pool = ctx.enter_context(tc.tile_pool(name="zsb", bufs=3))
        ln_pool = ctx.enter_context(tc.tile_pool(name="ln", bufs=3))
        lntmp_pool = ctx.enter_context(tc.tile_pool(name="lntmp", bufs=4))
        mbc_pool = ctx.enter_context(tc.tile_pool(name="mbc", bufs=3))
        big_pool = ctx.enter_context(tc.tile_pool(name="big", bufs=3))
        ws_pool = ctx.enter_context(tc.tile_pool(name="ws", bufs=1))
        wssq_pool = ctx.enter_context(tc.tile_pool(name="wssq", bufs=2))
        small_pool = ctx.enter_context(tc.tile_pool(name="small", bufs=4))
        out_pool = ctx.enter_context(tc.tile_pool(name="outp", bufs=3))

        psT_pool = ctx.enter_context(tc.tile_pool(name="psT", bufs=2, space="PSUM"))
        zps_pool = ctx.enter_context(tc.tile_pool(name="zps", bufs=2, space="PSUM"))
        stats_pool = ctx.enter_context(tc.tile_pool(name="stats", bufs=3, space="PSUM"))
        wsT_pool = ctx.enter_context(tc.tile_pool(name="wsT", bufs=1, space="PSUM"))

        def emit_chunk(e, c, y_sb):
            # mask broadcast tile for this chunk
            mb_ps = stats_pool.tile([128, 512], f32, tag="stats")
            nc.tensor.matmul(
                mb_ps[:],
                ones1b_sb[:],
                maskbf_sb[:, e * s + 512 * c : e * s + 512 * (c + 1)],
                start=True,
                stop=True,
            )
            mbc_sb = mbc_pool.tile([128, 512], f32, tag="mbc")
            nc.scalar.copy(mbc_sb[:], mb_ps[:])

            # load 4 s-tiles naturally
            hids = []
            for i in range(4):
                t = 4 * c + i
                ht = hid_pool.tile([128, H], f32r, tag="hid")
                nc.sync.dma_start(
                    out=ht[:], in_=hid_h.ap()[e, 128 * t : 128 * (t + 1), :]
                )
                hids.append(ht)

            # transpose to [h, s] and copy to SBUF
            hidTs = []
            for k in range(kh):
                ps = psT_pool.tile([128, 512], f32r, tag="psT")
                for i in range(4):
                    nc.tensor.transpose(
                        ps[:, 128 * i : 128 * (i + 1)],
                        hids[i][:, 128 * k : 128 * (k + 1)],
                        identr_sb[:],
                    )
                hT = hidT_pool.tile([128, 512], f32r, tag="hidT")
                if k % 3 != 1:
                    nc.scalar.copy(hT[:], ps[:])
                else:
                    nc.vector.tensor_copy(hT[:], ps[:])
                hidTs.append(hT)
                if c == 0:
                    # gather token-0 column for cls projection (full f32 from PSUM)
                    nc.vector.tensor_copy(
                        clsx_sb[:, k * bl + e : k * bl + e + 1], ps[:, 0:1]
                    )

            # projection: z[d, s]
            zps = zps_pool.tile([128, 512], f32, tag="zps")
            for k in range(kh):
                nc.tensor.matmul(
                    zps[:],
                    tokw_sb[k][:],
                    hidTs[k][:],
                    start=(k == 0),
                    stop=(k == kh - 1),
                )
            zsb = zsb_pool.tile([128, 512], f32r, tag="zsb")
            nc.scalar.copy(zsb[:], zps[:])

            # LN over d (partitions) via ones-matmul
            m1 = stats_pool.tile([128, 512], f32, tag="stats")
            nc.tensor.matmul(m1[:], ones_sb[:], zsb[:], start=True, stop=True)
            zc = ln_pool.tile([128, 512], f32, tag="zc")
            nc.vector.scalar_tensor_tensor(
                zc[:], m1[:], -1.0 / 128.0, zsb[:], OP.mult, OP.add
            )
            # q = zc * mask on GpSimd, in parallel with the var chain
            q = lntmp_pool.tile([128, 512], f32, tag="lnt")
            nc.gpsimd.tensor_mul(q[:], zc[:], mbc_sb[:])
            zcsq = lntmp_pool.tile([128, 512], f32r, tag="lnt")
            nc.scalar.square(zcsq[:], zc[:])
            m2 = stats_pool.tile([128, 512], f32, tag="stats")
            nc.tensor.matmul(m2[:], ones_sb[:], zcsq[:], start=True, stop=True)
            sd = lntmp_pool.tile([128, 512], f32, tag="lnt")
            nc.scalar.activation(sd[:], m2[:], AF.Sqrt, bias=eps_col[:, 0:1], scale=1.0 / 128.0)
            av = lntmp_pool.tile([128, 512], f32, tag="lnt")
            nc.vector.reciprocal(av[:], sd[:])
            t1 = ln_pool.tile([128, 512], f32, tag="t1")
            nc.vector.tensor_mul(t1[:], q[:], av[:])
            # y = relu(t1 * g)   (per-partition scale; ln_tok_b == 0 assumed)
            nc.scalar.activation(
                y_sb[:, 512 * c : 512 * (c + 1)], t1[:], AF.Relu, scale=gcol_sb[:, 0:1]
            )

        def emit_tail(e, y_sb):
            sc_sb = big_pool.tile([128, s + 10], f32, tag="SC")
            ws_sb = ws_pool.tile([128, s], f32, tag="WS")
            nc.gpsimd.memset(y_sb[:, s - 1 : s + 5], 0.0)
            nc.gpsimd.memset(sc_sb[:, 0:6], 0.0)
            nc.vector.tensor_tensor_scan(
                sc_sb[:, 6 : s + 10],
                y_sb[:, 1 : s + 5],
                y_sb[:, 1 : s + 5],
                0.0,
                OP.add,
                OP.bypass,
            )
            nc.gpsimd.tensor_sub(ws_sb[:], sc_sb[:, 10 : s + 10], sc_sb[:, 0:s])

            for g in range(ng):
                wsT = wsT_pool.tile([128, 512], f32, tag="wsT")
                for i in range(4):
                    t = 4 * g + i
                    nc.tensor.transpose(
                        wsT[:, 128 * i : 128 * (i + 1)],
                        ws_sb[:, 128 * t : 128 * (t + 1)],
                        ident_sb[:],
                    )
                wssq = wssq_pool.tile([128, 512], f32, tag="wssq")
                nsq = small_pool.tile([128, 4], f32, tag="nsq")
                for i in range(4):
                    nc.scalar.activation(
                        wssq[:, 128 * i : 128 * (i + 1)],
                        wsT[:, 128 * i : 128 * (i + 1)],
                        AF.Square,
                        accum_out=nsq[:, i : i + 1],
                    )
                sn = small_pool.tile([128, 4], f32, tag="sn")
                nc.scalar.activation(sn[:], nsq[:], AF.Sqrt, bias=tiny_col[:, 0:1])
                rn = small_pool.tile([128, 4], f32, tag="rn")
                nc.vector.reciprocal(rn[:], sn[:])
                fv = small_pool.tile([128, 4], f32, tag="fv")
                nc.vector.tensor_mul(
                    fv[:], rn[:], maskT_sb[:, e * nt + 4 * g : e * nt + 4 * g + 4]
                )
                osb = out_pool.tile([128, 512], f32, tag="outp")
                for i in range(4):
                    nc.vector.tensor_scalar(
                        osb[:, 128 * i : 128 * (i + 1)],
                        wsT[:, 128 * i : 128 * (i + 1)],
                        fv[:, i : i + 1],
                        None,
                        OP.mult,
                    )
                tok0 = 512 * g
                if g < ng - 1:
                    nc.sync.dma_start(
                        out=reps_out_h.ap()[e, tok0 : tok0 + 512, :].rearrange(
                            "(i p) d -> p i d", p=128
                        ),
                        in_=osb[:].rearrange("p (i d) -> p i d", d=128),
                    )
                else:
                    nc.sync.dma_start(
                        out=reps_out_h.ap()[e, tok0 : tok0 + 384, :].rearrange(
                            "(i p) d -> p i d", p=128
                        ),
                        in_=osb[:, 0:384].rearrange("p (i d) -> p i d", d=128),
                    )
                    nc.sync.dma_start(
                        out=reps_out_h.ap()[e, tok0 + 384 : t_len, :],
                        in_=osb[0:126, 384:512],
                    )

        pair_size = 2
        for p0 in range(0, bl, pair_size):
            pair = list(range(p0, min(p0 + pair_size, bl)))
            ys = {}
            for e in pair:
                ys[e] = big_pool.tile([128, s + 5], f32, tag="Y", name=f"Y{e}")
            for c in range(nch):
                for e in pair:
                    emit_chunk(e, c, ys[e])
            for e in pair:
                emit_tail(e, ys[e])

        # ---------- CLS head ----------
        cls_ps1 = stats_pool.tile([bl, 512], f32, tag="stats")
        cls_ps2 = stats_pool.tile([bl, CLS - 512], f32, tag="stats")
        for k in range(kh):
            nc.tensor.matmul(
                cls_ps1[:],
                clsx_sb[:, k * bl : (k + 1) * bl],
                clsw_sb[k][:, 0:512],
                start=(k == 0),
                stop=(k == kh - 1),
            )
        for k in range(kh):
            nc.tensor.matmul(
                cls_ps2[:],
                clsx_sb[:, k * bl : (k + 1) * bl],
                clsw_sb[k][:, 512:CLS],
                start=(k == 0),
                stop=(k == kh - 1),
            )
        cls_sb = cpool.tile([bl, CLS], f32, tag="cls_sb")
        nc.vector.scalar_tensor_tensor(
            cls_sb[:, 0:512], cls_ps1[:], 1.0, clsb_sb[:, 0:512], OP.mult, OP.add
        )
        nc.vector.scalar_tensor_tensor(
            cls_sb[:, 512:CLS], cls_ps2[:], 1.0, clsb_sb[:, 512:CLS], OP.mult, OP.add
        )
        csum = cpool.tile([bl, 1], f32, tag="csum")
        nc.vector.tensor_reduce(csum[:], cls_sb[:], mybir.AxisListType.X, OP.add)
        cmean = cpool.tile([bl, 1], f32, tag="cmean")
        nc.scalar.mul(cmean[:], csum[:], 1.0 / CLS)
        czc = cpool.tile([bl, CLS], f32, tag="czc")
        nc.vector.tensor_scalar(czc[:], cls_sb[:], cmean[:, 0:1], None, OP.subtract)
        csq = cpool.tile([bl, CLS], f32, tag="csq")
        nc.scalar.square(csq[:], czc[:])
        cssq = cpool.tile([bl, 1], f32, tag="cssq")
        nc.vector.tensor_reduce(cssq[:], csq[:], mybir.AxisListType.X, OP.add)
        csd = cpool.tile([bl, 1], f32, tag="csd")
        nc.scalar.activation(csd[:], cssq[:], AF.Sqrt, bias=eps_col[0:bl, 0:1], scale=1.0 / CLS)
        crs = cpool.tile([bl, 1], f32, tag="crs")
        nc.vector.reciprocal(crs[:], csd[:])
        cxn = cpool.tile([bl, CLS], f32, tag="cxn")
        nc.vector.tensor_scalar(cxn[:], czc[:], crs[:, 0:1], None, OP.mult)
        cg = cpool.tile([bl, CLS], f32, tag="cg")
        nc.vector.tensor_mul(cg[:], cxn[:], grep_sb[:])
        cfin = cpool.tile([bl, CLS], f32, tag="cfin")
        nc.vector.tensor_add(cfin[:], cg[:], brep_sb[:])
        nc.sync.dma_start(out=cls_out_h.ap(), in_=cfin[:])

    nc.finalize()
    return nc


def _host_prep(hidden, attention_mask, tok_w, tok_b, cls_w, cls_b,
               ln_tok_g, ln_tok_b, ln_cls_g, ln_cls_b, bl, s):
    """Build per-core input maps. hidden: [ncores*bl, s, H]."""
    t_len = s - 2
    nt = s // 128
    ncores = hidden.shape[0] // bl

    assert np.all(tok_b == 0.0), "kernel assumes tok_b == 0"
    assert np.all(ln_tok_b == 0.0), "kernel assumes ln_tok_b == 0"

    ident = np.eye(128, dtype=np.float32)
    ones128 = np.ones((128, 128), dtype=np.float32)
    ones1b = np.ones((1, 128), dtype=ml_dtypes.bfloat16)
    gcol = np.ascontiguousarray(ln_tok_g.reshape(128, 1).astype(np.float32))
    clsb_rep = np.tile(cls_b[None, :], (bl, 1)).astype(np.float32)
    g_rep = np.tile(ln_cls_g[None, :], (bl, 1)).astype(np.float32)
    b_rep = np.tile(ln_cls_b[None, :], (bl, 1)).astype(np.float32)
    tok_w = np.ascontiguousarray(tok_w.astype(np.float32))
    cls_w = np.ascontiguousarray(cls_w.astype(np.float32))

    in_maps = []
    for cid in range(ncores):
        hs = np.ascontiguousarray(hidden[cid * bl : (cid + 1) * bl].astype(np.float32))
        am = attention_mask[cid * bl : (cid + 1) * bl].astype(np.float32)  # [bl, s]
        maskbf = np.ascontiguousarray(am.reshape(1, bl * s).astype(ml_dtypes.bfloat16))
        # validity per compact token i (token j=i+1), zero-padded to s
        valid = np.zeros((bl, s), dtype=np.float32)
        valid[:, :t_len] = am[:, 1 : 1 + t_len]
        maskT = np.ascontiguousarray(
            np.concatenate(
                [valid[e].reshape(nt, 128).T for e in range(bl)], axis=1
            ).astype(np.float32)
        )  # [128, bl*nt]
        in_maps.append(
            dict(
                hidden=hs,
                maskbf=maskbf,
                maskT=maskT,
                tok_w=tok_w,
                g_col=gcol,
                cls_w=cls_w,
                clsb_rep=clsb_rep,
                g_rep=g_rep,
                b_rep=b_rep,
                ident=ident,
                identr=ident,
                ones128=ones128,
                ones1b=ones1b,
            )
        )
    return in_maps


_PROGRAM_CACHE = {}


def _get_program(bl, s):
    key = (bl, s)
    if key not in _PROGRAM_CACHE:
        _PROGRAM_CACHE[key] = build_program(bl, s)
    return _PROGRAM_CACHE[key]


def run(inputs, trace=False, trace_kwargs=None):
    """Execute on 8 cores; returns ((cls, reps), BassKernelResults)."""
    from concourse.bass_utils import run_bass_kernel_spmd

    nc = _get_program(BL, S_FULL)
    in_maps = _host_prep(
        np.asarray(inputs["hidden"]), np.asarray(inputs["attention_mask"]),
        np.asarray(inputs["tok_w"]), np.asarray(inputs["tok_b"]),
        np.asarray(inputs["cls_w"]), np.asarray(inputs["cls_b"]),
        np.asarray(inputs["ln_tok_g"]), np.asarray(inputs["ln_tok_b"]),
        np.asarray(inputs["ln_cls_g"]), np.asarray(inputs["ln_cls_b"]),
        BL, S_FULL,
    )
    kw = {}
    if trace:
        kw.update(trace=True, trace_kwargs=trace_kwargs or {})
    res = run_bass_kernel_spmd(nc, in_maps, core_ids=list(range(NCORES)), **kw)
    cls_rep = np.concatenate(
        [np.asarray(r["cls_out"]).reshape(BL, CLS) for r in res.results], axis=0
    )
    reps = np.concatenate(
        [np.asarray(r["reps_out"]).reshape(BL, S_FULL - 2, D) for r in res.results],
        axis=0,
    )
    return (cls_rep.astype(np.float32), reps.astype(np.float32)), res


def kernel(hidden, attention_mask, tok_w, tok_b, cls_w, cls_b,
           ln_tok_g, ln_tok_b, ln_cls_g, ln_cls_b):
    out, _ = run(
        dict(
            hidden=hidden, attention_mask=attention_mask, tok_w=tok_w, tok_b=tok_b,
            cls_w=cls_w, cls_b=cls_b, ln_tok_g=ln_tok_g, ln_tok_b=ln_tok_b,
            ln_cls_g=ln_cls_g, ln_cls_b=ln_cls_b,
        )
    )
    return out


# revision 19
# speedup vs baseline: 1.1504x; 1.1504x over previous
"""Trainium2 Bass kernel for COIL-style pooling head.

Computes, per example:
  cls_rep = LN(hidden[:,0] @ cls_w + cls_b) * g_cls + b_cls            [B, 768]
  reps    = relu(LN(hidden @ tok_w + tok_b) * g_tok + b_tok)           [B, S, 128]
  reps    = sliding-window (w=5) masked mean over compacted tokens 1..S-2,
            then L2-normalized                                          [B, S-2, 128]

Sharding: pure data parallel, batch 32 -> 4 examples on each of 8 cores.

Device pipeline per example (layout [d, s] = token-feature on partitions):
  1. DMA hidden s-tiles naturally [s,h], PE-transpose to [h,s] (fp32).
  2. fp32 matmul accumulation over 6 h-chunks -> z [d=128, s<=512] in PSUM.
  3. LN stats over d via ones-matmul (replicated column sums), center/scale,
     relu (*g on ACT with per-partition scale), mask multiply.
  4. One-op prefix scan along s (tensor_tensor_scan), shifted subtract ->
     10-wide window sums.
  5. PE-transpose back to [s, d], L2 normalize with safe divide + validity
     mask, DMA out.
"""

import sys

sys.path.insert(0, "/opt/trn_rl_repo")

from contextlib import ExitStack

import ml_dtypes
import numpy as np

import concourse.bass as bass
import concourse.tile as tile
from concourse import bacc
from concourse import mybir

B, S_FULL, H, D, CLS = 32, 2048, 768, 128, 768
WIN = 5
EPS = 1e-5
NCORES = 8
BL = B // NCORES  # examples per core

f32 = mybir.dt.float32
f32r = mybir.dt.float32r
bf16 = mybir.dt.bfloat16
AF = mybir.ActivationFunctionType
OP = mybir.AluOpType


def build_program(bl=BL, s=S_FULL):
    """Build the single-core Bass/Tile program (SPMD: same program on all cores)."""
    t_len = s - 2           # compact token count
    nt = s // 128           # s-tiles of 128 tokens
    nch = s // 512          # 512-wide chunks
    ng = nt // 4            # normalize groups of 4 s-tiles
    kh = H // 128           # h chunks (6)

    nc = bacc.Bacc("TRN2")

    # ---- DRAM parameters ----
    hid_h = nc.declare_dram_parameter("hidden", [bl, s, H], f32r, isOutput=False)
    maskbf_h = nc.declare_dram_parameter("maskbf", [1, bl * s], bf16, isOutput=False)
    maskT_h = nc.declare_dram_parameter("maskT", [128, bl * nt], f32, isOutput=False)
    tokw_h = nc.declare_dram_parameter("tok_w", [H, D], f32r, isOutput=False)
    gcol_h = nc.declare_dram_parameter("g_col", [128, 1], f32, isOutput=False)
    clsw_h = nc.declare_dram_parameter("cls_w", [H, CLS], f32, isOutput=False)
    clsb_h = nc.declare_dram_parameter("clsb_rep", [bl, CLS], f32, isOutput=False)
    grep_h = nc.declare_dram_parameter("g_rep", [bl, CLS], f32, isOutput=False)
    brep_h = nc.declare_dram_parameter("b_rep", [bl, CLS], f32, isOutput=False)
    ident_h = nc.declare_dram_parameter("ident", [128, 128], f32, isOutput=False)
    identr_h = nc.declare_dram_parameter("identr", [128, 128], f32r, isOutput=False)
    ones_h = nc.declare_dram_parameter("ones128", [128, 128], f32r, isOutput=False)
    ones1b_h = nc.declare_dram_parameter("ones1b", [1, 128], bf16, isOutput=False)

    cls_out_h = nc.declare_dram_parameter("cls_out", [bl, CLS], f32, isOutput=True)
    reps_out_h = nc.declare_dram_parameter("reps_out", [bl, t_len, D], f32, isOutput=True)

    with tile.TileContext(nc) as tc, ExitStack() as ctx:
        # ---- constant pools (resident) ----
        cpool = ctx.enter_context(tc.tile_pool(name="consts", bufs=1))

        ident_sb = cpool.tile([128, 128], f32, tag="ident")
        nc.sync.dma_start(out=ident_sb[:], in_=ident_h.ap())
        identr_sb = cpool.tile([128, 128], f32r, tag="identr")
        nc.sync.dma_start(out=identr_sb[:], in_=identr_h.ap())
        ones_sb = cpool.tile([128, 128], f32r, tag="ones")
        nc.sync.dma_start(out=ones_sb[:], in_=ones_h.ap())
        ones1b_sb = cpool.tile([1, 128], bf16, tag="ones1b")
        nc.sync.dma_start(out=ones1b_sb[:], in_=ones1b_h.ap())
        gcol_sb = cpool.tile([128, 1], f32, tag="gcol")
        nc.sync.dma_start(out=gcol_sb[:], in_=gcol_h.ap())
        maskbf_sb = cpool.tile([1, bl * s], bf16, tag="maskbf")
        nc.sync.dma_start(out=maskbf_sb[:], in_=maskbf_h.ap())
        maskT_sb = cpool.tile([128, bl * nt], f32, tag="maskT")
        nc.sync.dma_start(out=maskT_sb[:], in_=maskT_h.ap())

        tokw_sb = []
        for k in range(kh):
            w = cpool.tile([128, D], f32r, tag=f"tokw{k}")
            nc.sync.dma_start(out=w[:], in_=tokw_h.ap()[128 * k : 128 * (k + 1), :])
            tokw_sb.append(w)
        clsw_sb = []
        for k in range(kh):
            w = cpool.tile([128, CLS], f32, tag=f"clsw{k}")
            nc.sync.dma_start(out=w[:], in_=clsw_h.ap()[128 * k : 128 * (k + 1), :])
            clsw_sb.append(w)
        clsb_sb = cpool.tile([bl, CLS], f32, tag="clsb")
        nc.sync.dma_start(out=clsb_sb[:], in_=clsb_h.ap())
        grep_sb = cpool.tile([bl, CLS], f32, tag="grep")
        nc.sync.dma_start(out=grep_sb[:], in_=grep_h.ap())
        brep_sb = cpool.tile([bl, CLS], f32, tag="brep")
        nc.sync.dma_start(out=brep_sb[:], in_=brep_h.ap())

        # cls gather buffer: column k*bl+e = h-chunk k of token 0 of example e
        clsx_sb = cpool.tile([128, kh * bl], f32, tag="clsx")

        # bias constants for ACT ops
        eps_col = cpool.tile([128, 1], f32, tag="epsc")
        nc.gpsimd.memset(eps_col[:], EPS)
        tiny_col = cpool.tile([128, 1], f32, tag="tinyc")
        nc.gpsimd.memset(tiny_col[:], 1e-38)

        # ---- working pools ----
        hid_pool = ctx.enter_context(tc.tile_pool(name="hid", bufs=3))
        hidT_pool = ctx.enter_context(tc.tile_pool(name="hidT", bufs=10))
        zsb_pool = ctx.enter_context(tc.tile_pool(name="zsb", bufs=3))
        ln_pool = ctx.enter_context(tc.tile_pool(name="ln", bufs=3))
        lntmp_pool = ctx.enter_context(tc.tile_pool(name="lntmp", bufs=4))
        mbc_pool = ctx.enter_context(tc.tile_pool(name="mbc", bufs=3))
        big_pool = ctx.enter_context(tc.tile_pool(name="big", bufs=2))
        ws_pool = ctx.enter_context(tc.tile_pool(name="ws", bufs=1))
        wssq_pool = ctx.enter_context(tc.tile_pool(name="wssq", bufs=2))
        small_pool = ctx.enter_context(tc.tile_pool(name="small", bufs=4))
        out_pool = ctx.enter_context(tc.tile_pool(name="outp", bufs=4))

        psT_pool = ctx.enter_context(tc.tile_pool(name="psT", bufs=2, space="PSUM"))
        zps_pool = ctx.enter_context(tc.tile_pool(name="zps", bufs=2, space="PSUM"))
        stats_pool = ctx.enter_context(tc.tile_pool(name="stats", bufs=3, space="PSUM"))
        wsT_pool = ctx.enter_context(tc.tile_pool(name="wsT", bufs=1, space="PSUM"))

        def emit_chunk(e, c, y_sb):
            # mask broadcast tile for this chunk
            mb_ps = stats_pool.tile([128, 512], f32, tag="stats")
            nc.tensor.matmul(
                mb_ps[:],
                ones1b_sb[:],
                maskbf_sb[:, e * s + 512 * c : e * s + 512 * (c + 1)],
                start=True,
                stop=True,
            )
            mbc_sb = mbc_pool.tile([128, 512], f32, tag="mbc")
            nc.scalar.copy(mbc_sb[:], mb_ps[:])

            # load the whole 512-token chunk in one DMA
            hbig = hid_pool.tile([128, 4 * H], f32r, tag="hid")
            nc.sync.dma_start(
                out=hbig[:].rearrange("p (i h) -> p i h", h=H),
                in_=hid_h.ap()[e, 512 * c : 512 * (c + 1), :].rearrange(
                    "(i p) h -> p i h", p=128
                ),
            )
            hids = [hbig[:, i * H : (i + 1) * H] for i in range(4)]

            # transpose to [h, s] and copy to SBUF
            hidTs = []
            for k in range(kh):
                ps = psT_pool.tile([128, 512], f32r, tag="psT")
                for i in range(4):
                    nc.tensor.transpose(
                        ps[:, 128 * i : 128 * (i + 1)],
                        hids[i][:, 128 * k : 128 * (k + 1)],
                        identr_sb[:],
                    )
                nc.tensor.ldweights(ones1b_sb[:])
                hT = hidT_pool.tile([128, 512], f32r, tag="hidT")
                if k % 3 != 1:
                    nc.scalar.copy(hT[:], ps[:])
                else:
                    nc.vector.tensor_copy(hT[:], ps[:])
                hidTs.append(hT)
                if c == 0:
                    # gather token-0 column for cls projection (full f32 from PSUM)
                    nc.vector.tensor_copy(
                        clsx_sb[:, k * bl + e : k * bl + e + 1], ps[:, 0:1]
                    )

            # projection: z[d, s]
            zps = zps_pool.tile([128, 512], f32, tag="zps")
            for k in range(kh):
                nc.tensor.matmul(
                    zps[:],
                    tokw_sb[k][:],
                    hidTs[k][:],
                    start=(k == 0),
                    stop=(k == kh - 1),
                )
            zsb = zsb_pool.tile([128, 512], f32r, tag="zsb")
            nc.scalar.copy(zsb[:], zps[:])

            # LN over d (partitions) via ones-matmul
            m1 = stats_pool.tile([128, 512], f32, tag="stats")
            nc.tensor.matmul(m1[:], ones_sb[:], zsb[:], start=True, stop=True)
            zc = ln_pool.tile([128, 512], f32, tag="zc")
            nc.vector.scalar_tensor_tensor(
                zc[:], m1[:], -1.0 / 128.0, zsb[:], OP.mult, OP.add
            )
            # q = zc * mask on GpSimd, in parallel with the var chain
            q = lntmp_pool.tile([128, 512], f32, tag="lnt")
            nc.gpsimd.tensor_mul(q[:], zc[:], mbc_sb[:])
            zcsq = lntmp_pool.tile([128, 512], f32r, tag="lnt")
            nc.scalar.square(zcsq[:], zc[:])
            m2 = stats_pool.tile([128, 512], f32, tag="stats")
            nc.tensor.matmul(m2[:], ones_sb[:], zcsq[:], start=True, stop=True)
            sd = lntmp_pool.tile([128, 512], f32, tag="lnt")
            nc.scalar.activation(sd[:], m2[:], AF.Sqrt, bias=eps_col[:, 0:1], scale=1.0 / 128.0)
            av = lntmp_pool.tile([128, 512], f32, tag="lnt")
            nc.vector.reciprocal(av[:], sd[:])
            t1 = ln_pool.tile([128, 512], f32, tag="t1")
            nc.vector.tensor_mul(t1[:], q[:], av[:])
            # y = relu(t1 * g)   (per-partition scale; ln_tok_b == 0 assumed)
            nc.scalar.activation(
                y_sb[:, 512 * c : 512 * (c + 1)], t1[:], AF.Relu, scale=gcol_sb[:, 0:1]
            )

        def emit_tail(e, y_sb):
            sc_sb = big_pool.tile([128, s + 10], f32, tag="SC")
            ws_sb = ws_pool.tile([128, s], f32, tag="WS")
            nc.gpsimd.memset(y_sb[:, s - 1 : s + 5], 0.0)
            nc.gpsimd.memset(sc_sb[:, 0:6], 0.0)
            nc.vector.tensor_tensor_scan(
                sc_sb[:, 6 : s + 10],
                y_sb[:, 1 : s + 5],
                y_sb[:, 1 : s + 5],
                0.0,
                OP.add,
                OP.bypass,
            )
            nc.gpsimd.tensor_sub(ws_sb[:], sc_sb[:, 10 : s + 10], sc_sb[:, 0:s])

            for g in range(ng):
                wsT = wsT_pool.tile([128, 512], f32, tag="wsT")
                for i in range(4):
                    t = 4 * g + i
                    nc.tensor.transpose(
                        wsT[:, 128 * i : 128 * (i + 1)],
                        ws_sb[:, 128 * t : 128 * (t + 1)],
                        ident_sb[:],
                    )
                nc.tensor.ldweights(ones1b_sb[:])
                wssq = wssq_pool.tile([128, 512], f32, tag="wssq")
                nsq = small_pool.tile([128, 4], f32, tag="nsq")
                for i in range(4):
                    nc.scalar.activation(
                        wssq[:, 128 * i : 128 * (i + 1)],
                        wsT[:, 128 * i : 128 * (i + 1)],
                        AF.Square,
                        accum_out=nsq[:, i : i + 1],
                    )
                sn = small_pool.tile([128, 4], f32, tag="sn")
                nc.scalar.activation(sn[:], nsq[:], AF.Sqrt, bias=tiny_col[:, 0:1])
                rn = small_pool.tile([128, 4], f32, tag="rn")
                nc.vector.reciprocal(rn[:], sn[:])
                fv = small_pool.tile([128, 4], f32, tag="fv")
                nc.vector.tensor_mul(
                    fv[:], rn[:], maskT_sb[:, e * nt + 4 * g : e * nt + 4 * g + 4]
                )
                osb = out_pool.tile([128, 512], f32, tag="outp")
                for i in range(4):
                    nc.vector.tensor_scalar(
                        osb[:, 128 * i : 128 * (i + 1)],
                        wsT[:, 128 * i : 128 * (i + 1)],
                        fv[:, i : i + 1],
                        None,
                        OP.mult,
                    )
                tok0 = 512 * g
                if g < ng - 1:
                    nc.scalar.dma_start(
                        out=reps_out_h.ap()[e, tok0 : tok0 + 512, :].rearrange(
                            "(i p) d -> p i d", p=128
                        ),
                        in_=osb[:].rearrange("p (i d) -> p i d", d=128),
                    )
                else:
                    nc.scalar.dma_start(
                        out=reps_out_h.ap()[e, tok0 : tok0 + 384, :].rearrange(
                            "(i p) d -> p i d", p=128
                        ),
                        in_=osb[:, 0:384].rearrange("p (i d) -> p i d", d=128),
                    )
                    nc.scalar.dma_start(
                        out=reps_out_h.ap()[e, tok0 + 384 : t_len, :],
                        in_=osb[0:126, 384:512],
                    )

        pair_size = 2
        for p0 in range(0, bl, pair_size):
            pair = list(range(p0, min(p0 + pair_size, bl)))
            ys = {}
            for e in pair:
                ys[e] = big_pool.tile([128, s + 5], f32, tag="Y", name=f"Y{e}")
            for c in range(nch):
                for e in pair:
                    emit_chunk(e, c, ys[e])
            for e in pair:
                emit_tail(e, ys[e])

        # ---------- CLS head ----------
        cls_ps1 = stats_pool.tile([bl, 512], f32, tag="stats")
        cls_ps2 = stats_pool.tile([bl, CLS - 512], f32, tag="stats")
        for k in range(kh):
            nc.tensor.matmul(
                cls_ps1[:],
                clsx_sb[:, k * bl : (k + 1) * bl],
                clsw_sb[k][:, 0:512],
                start=(k == 0),
                stop=(k == kh - 1),
            )
        for k in range(kh):
            nc.tensor.matmul(
                cls_ps2[:],
                clsx_sb[:, k * bl : (k + 1) * bl],
                clsw_sb[k][:, 512:CLS],
                start=(k == 0),
                stop=(k == kh - 1),
            )
        cls_sb = cpool.tile([bl, CLS], f32, tag="cls_sb")
        nc.vector.scalar_tensor_tensor(
            cls_sb[:, 0:512], cls_ps1[:], 1.0, clsb_sb[:, 0:512], OP.mult, OP.add
        )
        nc.vector.scalar_tensor_tensor(
            cls_sb[:, 512:CLS], cls_ps2[:], 1.0, clsb_sb[:, 512:CLS], OP.mult, OP.add
        )
        csum = cpool.tile([bl, 1], f32, tag="csum")
        nc.vector.tensor_reduce(csum[:], cls_sb[:], mybir.AxisListType.X, OP.add)
        cmean = cpool.tile([bl, 1], f32, tag="cmean")
        nc.scalar.mul(cmean[:], csum[:], 1.0 / CLS)
        czc = cpool.tile([bl, CLS], f32, tag="czc")
        nc.vector.tensor_scalar(czc[:], cls_sb[:], cmean[:, 0:1], None, OP.subtract)
        csq = cpool.tile([bl, CLS], f32, tag="csq")
        nc.scalar.square(csq[:], czc[:])
        cssq = cpool.tile([bl, 1], f32, tag="cssq")
        nc.vector.tensor_reduce(cssq[:], csq[:], mybir.AxisListType.X, OP.add)
        csd = cpool.tile([bl, 1], f32, tag="csd")
        nc.scalar.activation(csd[:], cssq[:], AF.Sqrt, bias=eps_col[0:bl, 0:1], scale=1.0 / CLS)
        crs = cpool.tile([bl, 1], f32, tag="crs")
        nc.vector.reciprocal(crs[:], csd[:])
        cxn = cpool.tile([bl, CLS], f32, tag="cxn")
        nc.vector.tensor_scalar(cxn[:], czc[:], crs[:, 0:1], None, OP.mult)
        cg = cpool.tile([bl, CLS], f32, tag="cg")
        nc.vector.tensor_mul(cg[:], cxn[:], grep_sb[:])
        cfin = cpool.tile([bl, CLS], f32, tag="cfin")
        nc.vector.tensor_add(cfin[:], cg[:], brep_sb[:])
        nc.scalar.dma_start(out=cls_out_h.ap(), in_=cfin[:])

    nc.finalize()
    return nc


def _host_prep(hidden, attention_mask, tok_w, tok_b, cls_w, cls_b,
               ln_tok_g, ln_tok_b, ln_cls_g, ln_cls_b, bl, s):
    """Build per-core input maps. hidden: [ncores*bl, s, H]."""
    t_len = s - 2
    nt = s // 128
    ncores = hidden.shape[0] // bl

    assert np.all(tok_b == 0.0), "kernel assumes tok_b == 0"
    assert np.all(ln_tok_b == 0.0), "kernel assumes ln_tok_b == 0"

    ident = np.eye(128, dtype=np.float32)
    ones128 = np.ones((128, 128), dtype=np.float32)
    ones1b = np.ones((1, 128), dtype=ml_dtypes.bfloat16)
    gcol = np.ascontiguousarray(ln_tok_g.reshape(128, 1).astype(np.float32))
    clsb_rep = np.tile(cls_b[None, :], (bl, 1)).astype(np.float32)
    g_rep = np.tile(ln_cls_g[None, :], (bl, 1)).astype(np.float32)
    b_rep = np.tile(ln_cls_b[None, :], (bl, 1)).astype(np.float32)
    tok_w = np.ascontiguousarray(tok_w.astype(np.float32))
    cls_w = np.ascontiguousarray(cls_w.astype(np.float32))

    in_maps = []
    for cid in range(ncores):
        hs = np.ascontiguousarray(hidden[cid * bl : (cid + 1) * bl].astype(np.float32))
        am = attention_mask[cid * bl : (cid + 1) * bl].astype(np.float32)  # [bl, s]
        maskbf = np.ascontiguousarray(am.reshape(1, bl * s).astype(ml_dtypes.bfloat16))
        # validity per compact token i (token j=i+1), zero-padded to s
        valid = np.zeros((bl, s), dtype=np.float32)
        valid[:, :t_len] = am[:, 1 : 1 + t_len]
        maskT = np.ascontiguousarray(
            np.concatenate(
                [valid[e].reshape(nt, 128).T for e in range(bl)], axis=1
            ).astype(np.float32)
        )  # [128, bl*nt]
        in_maps.append(
            dict(
                hidden=hs,
                maskbf=maskbf,
                maskT=maskT,
                tok_w=tok_w,
                g_col=gcol,
                cls_w=cls_w,
                clsb_rep=clsb_rep,
                g_rep=g_rep,
                b_rep=b_rep,
                ident=ident,
                identr=ident,
                ones128=ones128,
                ones1b=ones1b,
            )
        )
    return in_maps


_PROGRAM_CACHE = {}


def _get_program(bl, s):
    key = (bl, s)
    if key not in _PROGRAM_CACHE:
        _PROGRAM_CACHE[key] = build_program(bl, s)
    return _PROGRAM_CACHE[key]


def run(inputs, trace=False, trace_kwargs=None):
    """Execute on 8 cores; returns ((cls, reps), BassKernelResults)."""
    from concourse.bass_utils import run_bass_kernel_spmd

    nc = _get_program(BL, S_FULL)
    in_maps = _host_prep(
        np.asarray(inputs["hidden"]), np.asarray(inputs["attention_mask"]),
        np.asarray(inputs["tok_w"]), np.asarray(inputs["tok_b"]),
        np.asarray(inputs["cls_w"]), np.asarray(inputs["cls_b"]),
        np.asarray(inputs["ln_tok_g"]), np.asarray(inputs["ln_tok_b"]),
        np.asarray(inputs["ln_cls_g"]), np.asarray(inputs["ln_cls_b"]),
        BL, S_FULL,
    )
    kw = {}
    if trace:
        kw.update(trace=True, trace_kwargs=trace_kwargs or {})
    res = run_bass_kernel_spmd(nc, in_maps, core_ids=list(range(NCORES)), **kw)
    cls_rep = np.concatenate(
        [np.asarray(r["cls_out"]).reshape(BL, CLS) for r in res.results], axis=0
    )
    reps = np.concatenate(
        [np.asarray(r["reps_out"]).reshape(BL, S_FULL - 2, D) for r in res.results],
        axis=0,
    )
    return (cls_rep.astype(np.float32), reps.astype(np.float32)), res


def kernel(hidden, attention_mask, tok_w, tok_b, cls_w, cls_b,
           ln_tok_g, ln_tok_b, ln_cls_g, ln_cls_b):
    out, _ = run(
        dict(
            hidden=hidden, attention_mask=attention_mask, tok_w=tok_w, tok_b=tok_b,
            cls_w=cls_w, cls_b=cls_b, ln_tok_g=ln_tok_g, ln_tok_b=ln_tok_b,
            ln_cls_g=ln_cls_g, ln_cls_b=ln_cls_b,
        )
    )
    return out
